# revision 28
# baseline (speedup 1.0000x reference)
"""EntropyLinear Trainium2 kernel (8-core SPMD, batch-sharded), v3.

Computes, for x[B,IN], weight[C,OUT,IN], bias[C,1,OUT]:
    gamma[c,i]      = sum_o |W[c,o,i]|
    alpha_norm[c,i] = exp((gamma[c,i] - max_i gamma[c,i]) / T)
    y[b,c,o]        = sum_i x[b,i] * alpha_norm[c,i] * W[c,o,i] + bias[c,0,o]

All-bf16 data path: x and W are cast to bf16 on the host and loaded
PRE-TRANSPOSED via the hardware DMA xbar-transpose (no PE operand
transposes, no PSUM evacuation for operands).  gamma comes straight off
the transposed W with abs-fused DVE reductions (no |W| pass, no mask
matmuls).  y is stored bf16 (half the store traffic) and upcast on the
host.  Bias is injected by rank-1 (ones x bias) matmuls for the
ACT-evacuated psum chunks and fused into the DVE add for the rest.

Loads/operands are split (W by co-piece, x by b-half, wm per chunk) so
downstream work starts as soon as its slice lands.  The repeat loop used
for timing is unrolled 8x inside the For_i (the Tile scheduler pipelines
the unrolled bodies; the loop back-edge is a full all-engine barrier, so
unrolling amortizes it and is what enables cross-iteration overlap) with
branch hints so the back-edge target stays in IRAM.  Staggered reset
(EL_STAGGER=1) was measured slower, and combining it with unroll
crashes the device -- keep EL_STAGGER=0.
"""

import os
from contextlib import ExitStack

import numpy as np
import ml_dtypes

import concourse.bass as bass
import concourse.tile as tile
from concourse import masks, mybir
from concourse.bass_utils import run_bass_kernel_spmd

# ---------------------------------------------------------------------------
# Workaround for this walrus build's 1-sync-wait-per-instruction limit:
# Tile's scheduler (and its exit drain) may attach several semaphore waits to
# one instruction; walrus codegen rejects that ("Too many sync wait
# commands").  Post-pass: hoist extra waits onto same-engine NOPs emitted
# immediately before the instruction (same engine stream => same semantics).
# ---------------------------------------------------------------------------


def _split_multi_waits(nc):
    for fn in nc.m.functions:
        for blk in fn.blocks:
            out = []
            changed = False
            for inst in list(blk.instructions):
                si = inst.sync_info
                waits = list(si.on_wait) if si is not None and si.on_wait else []
                if len(waits) > 1:
                    changed = True
                    for w in waits[:-1]:
                        nop = mybir.InstNoOp(
                            name=nc.get_next_instruction_name(), ins=[], outs=[])
                        nop.engine = inst.engine
                        nop.sync_info = mybir.SyncInfo(on_wait=[w], on_update=[])
                        nc.register_instruction(nop)
                        out.append(nop)
                    upd = list(si.on_update) if si.on_update else []
                    inst.sync_info = mybir.SyncInfo(
                        on_wait=[waits[-1]], on_update=upd)
                out.append(inst)
            if changed:
                blk.instructions = out

# ---------------------------------------------------------------------------

B, IN, OUT, C = 8192, 256, 32, 50
TEMPERATURE = 0.6
N_CORES = 8
BS = B // N_CORES          # rows of x per core
CO = C * OUT               # 1600 fused (class, out) columns
F32 = mybir.dt.float32
BF16 = mybir.dt.bfloat16

N_BT = BS // 128           # b-tiles per core (8)
N_KH = IN // 128           # contraction halves (2)
# W is loaded in two co-pieces split at class 32 (so the gamma transposes
# land at psum base partitions 0/32); psum chunks nest inside the pieces.
WCLS = (32, 18)                # classes per W co-piece
WC0 = (0, 32)                  # class base of each piece
WCOL0 = (0, 32 * OUT)          # column base of each piece (0, 1024)
CHUNK_NCLS = (16, 16, 9, 9)    # classes per psum chunk
CHUNK_C0 = (0, 16, 32, 41)
CHUNK_J = (0, 0, 1, 1)         # which W co-piece holds each chunk
CHUNK_COL0 = tuple(c * OUT for c in CHUNK_C0)          # (0, 512, 1024, 1312)
CHUNK_COLS = tuple(n * OUT for n in CHUNK_NCLS)        # (512, 512, 288, 288)
N_CC = len(CHUNK_NCLS)     # chunks per b-tile (4)
# chunks evacuated on ACT (bias via rank-1) vs DVE (bias fused in the add)
ACT_CHUNKS = tuple(
    int(s) for s in os.environ.get("EL_ACT", "0,2").split(",") if s != "")
DVE_CHUNKS = tuple(n for n in range(N_CC) if n not in ACT_CHUNKS)

BIAS_MODE = os.environ.get("EL_BIAS", "rank1")   # "rank1" | "gpsimd"
STAGGER = os.environ.get("EL_STAGGER", "0") == "1"
HINTS = os.environ.get("EL_HINTS", "1") == "1"
UNROLL = int(os.environ.get("EL_UNROLL", "16"))
# b-tile indices before which a staggered-reset stage boundary is placed
STAGES = tuple(int(s) for s in os.environ.get("EL_STAGES", "0,2,4").split(","))

_CACHE = {}


def _build(repeat=1):
    nc = bass.Bass(trn_type="TRN2", target_bir_lowering=False, debug=False,
                   num_devices=N_CORES)
    x_d = nc.dram_tensor("x", [BS, IN], BF16, kind="ExternalInput").ap()
    w_d = nc.dram_tensor("weight", [C, OUT, IN], BF16, kind="ExternalInput").ap()
    b_d = nc.dram_tensor("bias", [1, CO], BF16, kind="ExternalInput").ap()
    y_d = nc.dram_tensor("y", [BS, C, OUT], BF16, kind="ExternalOutput").ap()

    w_flat = w_d.rearrange("c o i -> (c o) i")      # [1600, 256]
    y_flat = y_d.rearrange("b c o -> b (c o)")      # [BS, 1600]

    BH = BS // 2            # b rows per x load piece (512)

    with tile.TileContext(nc) as tc, ExitStack() as ctx:
      NB = int(os.environ.get("EL_BUFS", "3"))
      const_p = ctx.enter_context(tc.tile_pool(name="const", bufs=NB))
      w_p = ctx.enter_context(tc.tile_pool(name="w", bufs=NB))
      x_p = ctx.enter_context(tc.tile_pool(name="x", bufs=NB))
      small_p = ctx.enter_context(tc.tile_pool(name="small", bufs=NB))
      y_p = ctx.enter_context(tc.tile_pool(name="y", bufs=6))
      ps_tp = ctx.enter_context(tc.tile_pool(name="ps_tp", bufs=1, space="PSUM"))
      ps_y = ctx.enter_context(tc.tile_pool(name="ps_y", bufs=6, space="PSUM"))

      hint_engines = ()
      if HINTS:
          hint_engines = (mybir.EngineType.PE, mybir.EngineType.DVE,
                          mybir.EngineType.Activation, mybir.EngineType.SP,
                          mybir.EngineType.Pool)
      unroll = UNROLL if repeat > 1 and repeat % UNROLL == 0 else 1
      rep_cm = (tc.For_i(0, repeat // unroll, 1, staggered_reset=STAGGER,
                         hint_engines=hint_engines)
                if repeat > 1 else None)
      if rep_cm is not None:
          rep_cm.__enter__()
      for _u in range(unroll if repeat > 1 else 1):
          # with stagger+unroll=4, the four stages are the four bodies
          if rep_cm is not None and STAGGER and unroll == 4 and _u > 0:
              tc.stage_boundary()
          # ---- constants ----
          ident = const_p.tile([128, 128], F32, tag="ident", name="ident")
          masks.make_identity(nc, ident[:])
          ones_r = const_p.tile([1, 128], BF16, tag="ones_r", name="ones_r")
          nc.vector.memset(ones_r[:], 1.0)

          # ---- loads (ACT HWDGE ring; stores go on the SP ring) ----
          # W transposed, split in co-pieces per i-half so gamma/wm start early
          wt = [[w_p.tile([128, WCLS[j] * OUT], BF16, tag=f"wt{h}{j}",
                          name=f"wt{h}{j}")
                 for j in range(2)] for h in range(N_KH)]
          for h in range(N_KH):
              for j in range(2):
                  nc.scalar.dma_start(
                      wt[h][j][:],
                      w_flat[WCOL0[j]:WCOL0[j] + WCLS[j] * OUT,
                             h * 128:(h + 1) * 128],
                      transpose=True)
          bias_r = const_p.tile([1, CO], BF16, tag="bias_r", name="bias_r")
          nc.scalar.dma_start(bias_r[:], b_d)
          # x transposed, split in b-halves
          xt = [[x_p.tile([128, BH], BF16, tag=f"xt{h}{j}", name=f"xt{h}{j}")
                 for j in range(2)] for h in range(N_KH)]
          for h in range(N_KH):
              for j in range(2):
                  nc.scalar.dma_start(
                      xt[h][j][:],
                      x_d[j * BH:(j + 1) * BH, h * 128:(h + 1) * 128],
                      transpose=True)

          def xsl(h, t):
              return xt[h][t // 4][:, (t % 4) * 128:(t % 4 + 1) * 128]

          # ---- prologue: gamma -> alpha -> wm ----
          # gamma quarters, i-major: gt[h][j][i, c'] = sum_o |wt[h][j]|
          gt = [[small_p.tile([128, WCLS[j]], F32, tag=f"gt{h}{j}",
                              name=f"gt{h}{j}")
                 for j in range(2)] for h in range(N_KH)]
          for h in range(N_KH):
              for j in range(2):
                  nc.vector.tensor_reduce(
                      gt[h][j][:],
                      wt[h][j][:].rearrange("p (c o) -> p c o", o=OUT),
                      axis=mybir.AxisListType.X, op=mybir.AluOpType.add,
                      apply_absolute_value=True)
          # transpose to rows [c', i] for the per-class max; transpose-mode
          # matmuls must write psum base partition 0, so each co-piece gets
          # its own row range packed into one [32, 512] psum bank
          grows2 = ps_tp.tile([32, 2 * IN], F32, tag="grows2", name="grows2")
          grows = [grows2[:WCLS[0], :IN], grows2[:WCLS[1], IN:]]
          for h in range(N_KH):
              for j in range(2):
                  nc.tensor.transpose(
                      grows[j][:, h * 128:(h + 1) * 128],
                      gt[h][j][:], ident[:])
          anp = [small_p.tile([WCLS[j], IN], F32, tag=f"anp{j}",
                              name=f"anp{j}") for j in range(2)]
          for j in range(2):
              gm = small_p.tile([WCLS[j], 1], F32, tag=f"gm{j}",
                                name=f"gm{j}")
              nc.vector.tensor_reduce(gm[:], grows[j][:],
                                      axis=mybir.AxisListType.X,
                                      op=mybir.AluOpType.max)
              nb = small_p.tile([WCLS[j], 1], F32, tag=f"nb{j}",
                                name=f"nb{j}")
              nc.vector.tensor_scalar_mul(nb[:], gm[:], -1.0 / TEMPERATURE)
              nc.scalar.activation(anp[j][:], grows[j][:],
                                   mybir.ActivationFunctionType.Exp,
                                   bias=nb[:], scale=1.0 / TEMPERATURE)
          # back to i-major: ant[h][i, c], bf16
          antp = ps_tp.tile([128, 2 * C], F32, tag="antp", name="antp")
          for h in range(N_KH):
              for j in range(2):
                  nc.tensor.transpose(
                      antp[:, h * C + WC0[j]:h * C + WC0[j] + WCLS[j]],
                      anp[j][:, h * 128:(h + 1) * 128],
                      ident[:WCLS[j], :WCLS[j]])
          ant = [small_p.tile([128, C], BF16, tag=f"ant{h}", name=f"ant{h}")
                 for h in range(N_KH)]
          for h in range(N_KH):
              nc.vector.tensor_copy(ant[h][:], antp[:, h * C:(h + 1) * C])

          # wm per (h, chunk): wm[h][n][i, co'] = wt * ant (bcast over o)
          wm = [[w_p.tile([128, CHUNK_COLS[n]], BF16, tag=f"wm{h}{n}",
                          name=f"wm{h}{n}")
                 for n in range(N_CC)] for h in range(N_KH)]
          for h in range(N_KH):
              for n in range(N_CC):
                  j = CHUNK_J[n]                     # wt co-piece of chunk n
                  base = CHUNK_COL0[n] - WCOL0[j]    # col offset in the piece
                  c0, ncls = CHUNK_C0[n], CHUNK_NCLS[n]
                  nc.vector.tensor_tensor(
                      wm[h][n][:].rearrange("p (c o) -> p c o", o=OUT),
                      wt[h][j][:, base:base + CHUNK_COLS[n]].rearrange(
                          "p (c o) -> p c o", o=OUT),
                      ant[h][:, c0:c0 + ncls].unsqueeze(2).broadcast_to(
                          [128, ncls, OUT]),
                      op=mybir.AluOpType.mult)

          # bias_rep for the DVE-evacuated chunk columns (f32) and, in
          # gpsimd mode, for the ACT-evacuated columns too (bf16, added
          # in-place by the Pool engine instead of PE rank-1 matmuls)
          nrep = sum(CHUNK_COLS[n] for n in DVE_CHUNKS)
          bias_rep = const_p.tile([128, max(1, nrep)],
                                  F32, tag="bias_rep", name="bias_rep")
          dcol = 0
          for n in DVE_CHUNKS:
              sl = slice(CHUNK_COL0[n], CHUNK_COL0[n] + CHUNK_COLS[n])
              dsl = slice(dcol, dcol + CHUNK_COLS[n])
              dcol += CHUNK_COLS[n]
              ps = ps_y.tile([128, 512], F32, tag="ps", name="ps")
              nc.tensor.matmul(ps[:, :CHUNK_COLS[n]], ones_r[:], bias_r[:, sl],
                               start=True, stop=True)
              nc.scalar.copy(bias_rep[:, dsl], ps[:, :CHUNK_COLS[n]])
          if BIAS_MODE == "gpsimd":
              ngrep = sum(CHUNK_COLS[n] for n in ACT_CHUNKS)
              bias_rep_g = const_p.tile([128, ngrep], BF16, tag="bias_rep_g",
                                        name="bias_rep_g")
              gcol = 0
              for n in ACT_CHUNKS:
                  sl = slice(CHUNK_COL0[n], CHUNK_COL0[n] + CHUNK_COLS[n])
                  gsl = slice(gcol, gcol + CHUNK_COLS[n])
                  gcol += CHUNK_COLS[n]
                  ps = ps_y.tile([128, 512], F32, tag="ps", name="ps")
                  nc.tensor.matmul(ps[:, :CHUNK_COLS[n]], ones_r[:],
                                   bias_r[:, sl], start=True, stop=True)
                  nc.scalar.copy(bias_rep_g[:, gsl], ps[:, :CHUNK_COLS[n]])

          # ---- main loop over b-tiles ----
          for t in range(N_BT):
              if (rep_cm is not None and STAGGER and unroll == 1
                      and t in STAGES):
                  tc.stage_boundary()
              y_sb = y_p.tile([128, CO], BF16, tag="y_sb", name="y_sb")
              pss = []
              for n in range(N_CC):
                  sl = slice(CHUNK_COL0[n], CHUNK_COL0[n] + CHUNK_COLS[n])
                  ps = ps_y.tile([128, 512], F32, tag="ps", name="ps")
                  pss.append(ps[:, :CHUNK_COLS[n]])
                  if n in ACT_CHUNKS and BIAS_MODE == "rank1":
                      nc.tensor.matmul(pss[n], ones_r[:], bias_r[:, sl],
                                       start=True, stop=False)
              for h in range(N_KH):
                  for n in range(N_CC):
                      start = (h == 0 and (n in DVE_CHUNKS
                                           or BIAS_MODE == "gpsimd"))
                      nc.tensor.matmul(pss[n], xsl(h, t), wm[h][n][:],
                                       start=start, stop=(h == N_KH - 1))
              dcol = 0
              for n in range(N_CC):
                  sl = slice(CHUNK_COL0[n], CHUNK_COL0[n] + CHUNK_COLS[n])
                  if n in ACT_CHUNKS:
                      nc.scalar.copy(y_sb[:, sl], pss[n])
                  else:
                      dsl = slice(dcol, dcol + CHUNK_COLS[n])
                      dcol += CHUNK_COLS[n]
                      nc.vector.tensor_tensor(y_sb[:, sl], pss[n],
                                              bias_rep[:, dsl],
                                              op=mybir.AluOpType.add)
              if BIAS_MODE == "gpsimd":
                  # ACT chunks are contiguous columns 0:1024 in this layout
                  gcol = 0
                  for n in ACT_CHUNKS:
                      sl = slice(CHUNK_COL0[n], CHUNK_COL0[n] + CHUNK_COLS[n])
                      gsl = slice(gcol, gcol + CHUNK_COLS[n])
                      gcol += CHUNK_COLS[n]
                      nc.gpsimd.tensor_tensor(y_sb[:, sl], y_sb[:, sl],
                                              bias_rep_g[:, gsl],
                                              op=mybir.AluOpType.add)
              nc.sync.dma_start(y_flat[t * 128:(t + 1) * 128, :], y_sb[:])

      if rep_cm is not None:
          rep_cm.__exit__(None, None, None)

    _split_multi_waits(nc)
    return nc


def _get_nc(repeat=1):
    if repeat not in _CACHE:
        _CACHE[repeat] = _build(repeat)
    return _CACHE[repeat]


def kernel(x: np.ndarray, weight: np.ndarray, bias: np.ndarray,
           _trace: bool = False, _repeat: int = 1):
    nc = _get_nc(_repeat)
    xb = np.ascontiguousarray(x, dtype=np.float32).astype(ml_dtypes.bfloat16)
    wb = np.ascontiguousarray(weight, dtype=np.float32).astype(ml_dtypes.bfloat16)
    bb = np.ascontiguousarray(
        bias, dtype=np.float32).reshape(1, CO).astype(ml_dtypes.bfloat16)
    in_maps = [
        {"x": xb[i * BS:(i + 1) * BS], "weight": wb, "bias": bb}
        for i in range(N_CORES)
    ]
    res = run_bass_kernel_spmd(nc, in_maps, list(range(N_CORES)), trace=_trace)
    out = np.concatenate(
        [np.asarray(res.results[i]["y"]) for i in range(N_CORES)],
        axis=0).astype(np.float32)
    if _trace:
        return out, res
    return out


# revision 31
# speedup vs baseline: 1.2233x; 1.2233x over previous
"""EntropyLinear Trainium2 kernel (8-core SPMD, batch-sharded), v3.

Computes, for x[B,IN], weight[C,OUT,IN], bias[C,1,OUT]:
    gamma[c,i]      = sum_o |W[c,o,i]|
    alpha_norm[c,i] = exp((gamma[c,i] - max_i gamma[c,i]) / T)
    y[b,c,o]        = sum_i x[b,i] * alpha_norm[c,i] * W[c,o,i] + bias[c,0,o]

All-bf16 data path: x and W are cast to bf16 on the host and loaded
PRE-TRANSPOSED via the hardware DMA xbar-transpose (no PE operand
transposes, no PSUM evacuation for operands).  gamma comes straight off
the transposed W with abs-fused DVE reductions (no |W| pass, no mask
matmuls).  y is stored bf16 (half the store traffic) and upcast on the
host.  Bias is injected by rank-1 (ones x bias) matmuls for the
ACT-evacuated psum chunks and fused into the DVE add for the rest.

Loads/operands are split (W by co-piece, x by b-half, wm per chunk) so
downstream work starts as soon as its slice lands.  The repeat loop used
for timing is unrolled 8x inside the For_i (the Tile scheduler pipelines
the unrolled bodies; the loop back-edge is a full all-engine barrier, so
unrolling amortizes it and is what enables cross-iteration overlap) with
branch hints so the back-edge target stays in IRAM.  Staggered reset
(EL_STAGGER=1) was measured slower, and combining it with unroll
crashes the device -- keep EL_STAGGER=0.
"""

import os
from contextlib import ExitStack

import numpy as np
import ml_dtypes

import concourse.bass as bass
import concourse.tile as tile
from concourse import masks, mybir
from concourse.bass_utils import run_bass_kernel_spmd

# ---------------------------------------------------------------------------
# Workaround for this walrus build's 1-sync-wait-per-instruction limit:
# Tile's scheduler (and its exit drain) may attach several semaphore waits to
# one instruction; walrus codegen rejects that ("Too many sync wait
# commands").  Post-pass: hoist extra waits onto same-engine NOPs emitted
# immediately before the instruction (same engine stream => same semantics).
# ---------------------------------------------------------------------------


def _split_multi_waits(nc):
    for fn in nc.m.functions:
        for blk in fn.blocks:
            out = []
            changed = False
            for inst in list(blk.instructions):
                si = inst.sync_info
                waits = list(si.on_wait) if si is not None and si.on_wait else []
                if len(waits) > 1:
                    changed = True
                    for w in waits[:-1]:
                        nop = mybir.InstNoOp(
                            name=nc.get_next_instruction_name(), ins=[], outs=[])
                        nop.engine = inst.engine
                        nop.sync_info = mybir.SyncInfo(on_wait=[w], on_update=[])
                        nc.register_instruction(nop)
                        out.append(nop)
                    upd = list(si.on_update) if si.on_update else []
                    inst.sync_info = mybir.SyncInfo(
                        on_wait=[waits[-1]], on_update=upd)
                out.append(inst)
            if changed:
                blk.instructions = out

# ---------------------------------------------------------------------------

B, IN, OUT, C = 8192, 256, 32, 50
TEMPERATURE = 0.6
N_CORES = 8
BS = B // N_CORES          # rows of x per core
CO = C * OUT               # 1600 fused (class, out) columns
F32 = mybir.dt.float32
BF16 = mybir.dt.bfloat16

N_BT = BS // 128           # b-tiles per core (8)
N_KH = IN // 128           # contraction halves (2)
# W is loaded in two co-pieces split at class 32 (so the gamma transposes
# land at psum base partitions 0/32); psum chunks nest inside the pieces.
WCLS = (32, 18)                # classes per W co-piece
WC0 = (0, 32)                  # class base of each piece
WCOL0 = (0, 32 * OUT)          # column base of each piece (0, 1024)
CHUNK_NCLS = (16, 16, 9, 9)    # classes per psum chunk
CHUNK_C0 = (0, 16, 32, 41)
CHUNK_J = (0, 0, 1, 1)         # which W co-piece holds each chunk
CHUNK_COL0 = tuple(c * OUT for c in CHUNK_C0)          # (0, 512, 1024, 1312)
CHUNK_COLS = tuple(n * OUT for n in CHUNK_NCLS)        # (512, 512, 288, 288)
N_CC = len(CHUNK_NCLS)     # chunks per b-tile (4)
# chunks evacuated on ACT (bias via rank-1) vs DVE (bias fused in the add)
ACT_CHUNKS = tuple(
    int(s) for s in os.environ.get("EL_ACT", "0,2").split(",") if s != "")
DVE_CHUNKS = tuple(n for n in range(N_CC) if n not in ACT_CHUNKS)

BIAS_MODE = os.environ.get("EL_BIAS", "rank1")   # "rank1" | "gpsimd"
COARSE = os.environ.get("EL_COARSE", "0") == "1"  # unsplit W load + gamma
STAGGER = os.environ.get("EL_STAGGER", "0") == "1"
HINTS = os.environ.get("EL_HINTS", "1") == "1"
UNROLL = int(os.environ.get("EL_UNROLL", "16"))
# b-tile indices before which a staggered-reset stage boundary is placed
STAGES = tuple(int(s) for s in os.environ.get("EL_STAGES", "0,2,4").split(","))

_CACHE = {}


def _build(repeat=1):
    nc = bass.Bass(trn_type="TRN2", target_bir_lowering=False, debug=False,
                   num_devices=N_CORES)
    x_d = nc.dram_tensor("x", [BS, IN], BF16, kind="ExternalInput").ap()
    w_d = nc.dram_tensor("weight", [C, OUT, IN], BF16, kind="ExternalInput").ap()
    b_d = nc.dram_tensor("bias", [1, CO], BF16, kind="ExternalInput").ap()
    y_d = nc.dram_tensor("y", [BS, C, OUT], BF16, kind="ExternalOutput").ap()

    w_flat = w_d.rearrange("c o i -> (c o) i")      # [1600, 256]
    y_flat = y_d.rearrange("b c o -> b (c o)")      # [BS, 1600]

    BH = BS // 2            # b rows per x load piece (512)

    with tile.TileContext(nc) as tc, ExitStack() as ctx:
      NB = int(os.environ.get("EL_BUFS", "3"))
      const_p = ctx.enter_context(tc.tile_pool(name="const", bufs=NB))
      w_p = ctx.enter_context(tc.tile_pool(name="w", bufs=NB))
      x_p = ctx.enter_context(tc.tile_pool(name="x", bufs=NB))
      small_p = ctx.enter_context(tc.tile_pool(name="small", bufs=NB))
      y_p = ctx.enter_context(tc.tile_pool(name="y", bufs=6))
      ps_tp = ctx.enter_context(tc.tile_pool(name="ps_tp", bufs=1, space="PSUM"))
      ps_y = ctx.enter_context(tc.tile_pool(name="ps_y", bufs=6, space="PSUM"))

      hint_engines = ()
      if HINTS:
          hint_engines = (mybir.EngineType.PE, mybir.EngineType.DVE,
                          mybir.EngineType.Activation, mybir.EngineType.SP,
                          mybir.EngineType.Pool)
      unroll = UNROLL if repeat > 1 and repeat % UNROLL == 0 else 1
      rep_cm = (tc.For_i(0, repeat // unroll, 1, staggered_reset=STAGGER,
                         hint_engines=hint_engines)
                if repeat > 1 else None)
      if rep_cm is not None:
          rep_cm.__enter__()
      for _u in range(unroll if repeat > 1 else 1):
          # with stagger+unroll=4, the four stages are the four bodies
          if rep_cm is not None and STAGGER and unroll == 4 and _u > 0:
              tc.stage_boundary()
          # ---- constants ----
          ident = const_p.tile([128, 128], F32, tag="ident", name="ident")
          masks.make_identity(nc, ident[:])
          ones_r = const_p.tile([1, 128], BF16, tag="ones_r", name="ones_r")
          nc.vector.memset(ones_r[:], 1.0)

          # ---- loads (ACT HWDGE ring; stores go on the SP ring) ----
          # W transposed; either one piece per i-half (coarse) or split in
          # co-pieces so gamma/wm start earlier within a body
          if COARSE:
              wtf = [w_p.tile([128, CO], BF16, tag=f"wt{h}", name=f"wt{h}")
                     for h in range(N_KH)]
              for h in range(N_KH):
                  nc.scalar.dma_start(wtf[h][:],
                                      w_flat[:, h * 128:(h + 1) * 128],
                                      transpose=True)
              wt = [[wtf[h][:, WCOL0[j]:WCOL0[j] + WCLS[j] * OUT]
                     for j in range(2)] for h in range(N_KH)]
          else:
              wt = [[w_p.tile([128, WCLS[j] * OUT], BF16, tag=f"wt{h}{j}",
                              name=f"wt{h}{j}")
                     for j in range(2)] for h in range(N_KH)]
              for h in range(N_KH):
                  for j in range(2):
                      nc.scalar.dma_start(
                          wt[h][j][:],
                          w_flat[WCOL0[j]:WCOL0[j] + WCLS[j] * OUT,
                                 h * 128:(h + 1) * 128],
                          transpose=True)
          bias_r = const_p.tile([1, CO], BF16, tag="bias_r", name="bias_r")
          nc.scalar.dma_start(bias_r[:], b_d)
          # x transposed, split in b-halves
          xt = [[x_p.tile([128, BH], BF16, tag=f"xt{h}{j}", name=f"xt{h}{j}")
                 for j in range(2)] for h in range(N_KH)]
          for h in range(N_KH):
              for j in range(2):
                  nc.scalar.dma_start(
                      xt[h][j][:],
                      x_d[j * BH:(j + 1) * BH, h * 128:(h + 1) * 128],
                      transpose=True)

          def xsl(h, t):
              return xt[h][t // 4][:, (t % 4) * 128:(t % 4 + 1) * 128]

          # ---- prologue: gamma -> alpha -> wm ----
          antp = ps_tp.tile([128, 2 * C], F32, tag="antp", name="antp")
          if COARSE:
              # one abs-reduce + one row-transpose per i-half, single exp
              gtf = [small_p.tile([128, C], F32, tag=f"gt{h}", name=f"gt{h}")
                     for h in range(N_KH)]
              for h in range(N_KH):
                  nc.vector.tensor_reduce(
                      gtf[h][:],
                      wtf[h][:].rearrange("p (c o) -> p c o", o=OUT),
                      axis=mybir.AxisListType.X, op=mybir.AluOpType.add,
                      apply_absolute_value=True)
              growsf = ps_tp.tile([C, IN], F32, tag="grows2", name="grows2")
              for h in range(N_KH):
                  nc.tensor.transpose(growsf[:, h * 128:(h + 1) * 128],
                                      gtf[h][:], ident[:])
              gmf = small_p.tile([C, 1], F32, tag="gm0", name="gm0")
              nc.vector.tensor_reduce(gmf[:], growsf[:],
                                      axis=mybir.AxisListType.X,
                                      op=mybir.AluOpType.max)
              nbf = small_p.tile([C, 1], F32, tag="nb0", name="nb0")
              nc.vector.tensor_scalar_mul(nbf[:], gmf[:], -1.0 / TEMPERATURE)
              anpf = small_p.tile([C, IN], F32, tag="anp0", name="anp0")
              nc.scalar.activation(anpf[:], growsf[:],
                                   mybir.ActivationFunctionType.Exp,
                                   bias=nbf[:], scale=1.0 / TEMPERATURE)
              for h in range(N_KH):
                  nc.tensor.transpose(antp[:, h * C:(h + 1) * C],
                                      anpf[:, h * 128:(h + 1) * 128],
                                      ident[:C, :C])
          else:
              # gamma quarters, i-major: gt[h][j][i, c'] = sum_o |wt[h][j]|
              gt = [[small_p.tile([128, WCLS[j]], F32, tag=f"gt{h}{j}",
                                  name=f"gt{h}{j}")
                     for j in range(2)] for h in range(N_KH)]
              for h in range(N_KH):
                  for j in range(2):
                      nc.vector.tensor_reduce(
                          gt[h][j][:],
                          wt[h][j][:].rearrange("p (c o) -> p c o", o=OUT),
                          axis=mybir.AxisListType.X, op=mybir.AluOpType.add,
                          apply_absolute_value=True)
              # transpose to rows [c', i] for the per-class max;
              # transpose-mode matmuls must write psum base partition 0, so
              # each co-piece gets its own row range in one [32, 512] bank
              grows2 = ps_tp.tile([32, 2 * IN], F32, tag="grows2",
                                  name="grows2")
              grows = [grows2[:WCLS[0], :IN], grows2[:WCLS[1], IN:]]
              for h in range(N_KH):
                  for j in range(2):
                      nc.tensor.transpose(
                          grows[j][:, h * 128:(h + 1) * 128],
                          gt[h][j][:], ident[:])
              anp = [small_p.tile([WCLS[j], IN], F32, tag=f"anp{j}",
                                  name=f"anp{j}") for j in range(2)]
              for j in range(2):
                  gm = small_p.tile([WCLS[j], 1], F32, tag=f"gm{j}",
                                    name=f"gm{j}")
                  nc.vector.tensor_reduce(gm[:], grows[j][:],
                                          axis=mybir.AxisListType.X,
                                          op=mybir.AluOpType.max)
                  nb = small_p.tile([WCLS[j], 1], F32, tag=f"nb{j}",
                                    name=f"nb{j}")
                  nc.vector.tensor_scalar_mul(nb[:], gm[:],
                                              -1.0 / TEMPERATURE)
                  nc.scalar.activation(anp[j][:], grows[j][:],
                                       mybir.ActivationFunctionType.Exp,
                                       bias=nb[:], scale=1.0 / TEMPERATURE)
              # back to i-major: ant[h][i, c], bf16
              for h in range(N_KH):
                  for j in range(2):
                      nc.tensor.transpose(
                          antp[:, h * C + WC0[j]:h * C + WC0[j] + WCLS[j]],
                          anp[j][:, h * 128:(h + 1) * 128],
                          ident[:WCLS[j], :WCLS[j]])
          ant = [small_p.tile([128, C], BF16, tag=f"ant{h}", name=f"ant{h}")
                 for h in range(N_KH)]
          for h in range(N_KH):
              nc.vector.tensor_copy(ant[h][:], antp[:, h * C:(h + 1) * C])

          # wm per (h, chunk): wm[h][n][i, co'] = wt * ant (bcast over o)
          wm = [[w_p.tile([128, CHUNK_COLS[n]], BF16, tag=f"wm{h}{n}",
                          name=f"wm{h}{n}")
                 for n in range(N_CC)] for h in range(N_KH)]
          for h in range(N_KH):
              for n in range(N_CC):
                  j = CHUNK_J[n]                     # wt co-piece of chunk n
                  base = CHUNK_COL0[n] - WCOL0[j]    # col offset in the piece
                  c0, ncls = CHUNK_C0[n], CHUNK_NCLS[n]
                  nc.vector.tensor_tensor(
                      wm[h][n][:].rearrange("p (c o) -> p c o", o=OUT),
                      wt[h][j][:, base:base + CHUNK_COLS[n]].rearrange(
                          "p (c o) -> p c o", o=OUT),
                      ant[h][:, c0:c0 + ncls].unsqueeze(2).broadcast_to(
                          [128, ncls, OUT]),
                      op=mybir.AluOpType.mult)

          # bias_rep for the DVE-evacuated chunk columns (f32) and, in
          # gpsimd mode, for the ACT-evacuated columns too (bf16, added
          # in-place by the Pool engine instead of PE rank-1 matmuls)
          nrep = sum(CHUNK_COLS[n] for n in DVE_CHUNKS)
          bias_rep = const_p.tile([128, max(1, nrep)],
                                  F32, tag="bias_rep", name="bias_rep")
          dcol = 0
          for n in DVE_CHUNKS:
              sl = slice(CHUNK_COL0[n], CHUNK_COL0[n] + CHUNK_COLS[n])
              dsl = slice(dcol, dcol + CHUNK_COLS[n])
              dcol += CHUNK_COLS[n]
              ps = ps_y.tile([128, 512], F32, tag="ps", name="ps")
              nc.tensor.matmul(ps[:, :CHUNK_COLS[n]], ones_r[:], bias_r[:, sl],
                               start=True, stop=True)
              nc.scalar.copy(bias_rep[:, dsl], ps[:, :CHUNK_COLS[n]])
          if BIAS_MODE == "gpsimd":
              ngrep = sum(CHUNK_COLS[n] for n in ACT_CHUNKS)
              bias_rep_g = const_p.tile([128, ngrep], BF16, tag="bias_rep_g",
                                        name="bias_rep_g")
              gcol = 0
              for n in ACT_CHUNKS:
                  sl = slice(CHUNK_COL0[n], CHUNK_COL0[n] + CHUNK_COLS[n])
                  gsl = slice(gcol, gcol + CHUNK_COLS[n])
                  gcol += CHUNK_COLS[n]
                  ps = ps_y.tile([128, 512], F32, tag="ps", name="ps")
                  nc.tensor.matmul(ps[:, :CHUNK_COLS[n]], ones_r[:],
                                   bias_r[:, sl], start=True, stop=True)
                  nc.scalar.copy(bias_rep_g[:, gsl], ps[:, :CHUNK_COLS[n]])

          # ---- main loop over b-tiles ----
          for t in range(N_BT):
              if (rep_cm is not None and STAGGER and unroll == 1
                      and t in STAGES):
                  tc.stage_boundary()
              y_sb = y_p.tile([128, CO], BF16, tag="y_sb", name="y_sb")
              pss = []
              for n in range(N_CC):
                  sl = slice(CHUNK_COL0[n], CHUNK_COL0[n] + CHUNK_COLS[n])
                  ps = ps_y.tile([128, 512], F32, tag="ps", name="ps")
                  pss.append(ps[:, :CHUNK_COLS[n]])
                  if n in ACT_CHUNKS and BIAS_MODE == "rank1":
                      nc.tensor.matmul(pss[n], ones_r[:], bias_r[:, sl],
                                       start=True, stop=False)
              for h in range(N_KH):
                  for n in range(N_CC):
                      start = (h == 0 and (n in DVE_CHUNKS
                                           or BIAS_MODE == "gpsimd"))
                      nc.tensor.matmul(pss[n], xsl(h, t), wm[h][n][:],
                                       start=start, stop=(h == N_KH - 1))
              dcol = 0
              for n in range(N_CC):
                  sl = slice(CHUNK_COL0[n], CHUNK_COL0[n] + CHUNK_COLS[n])
                  if n in ACT_CHUNKS:
                      nc.scalar.copy(y_sb[:, sl], pss[n])
                  else:
                      dsl = slice(dcol, dcol + CHUNK_COLS[n])
                      dcol += CHUNK_COLS[n]
                      nc.vector.tensor_tensor(y_sb[:, sl], pss[n],
                                              bias_rep[:, dsl],
                                              op=mybir.AluOpType.add)
              if BIAS_MODE == "gpsimd":
                  # ACT chunks are contiguous columns 0:1024 in this layout
                  gcol = 0
                  for n in ACT_CHUNKS:
                      sl = slice(CHUNK_COL0[n], CHUNK_COL0[n] + CHUNK_COLS[n])
                      gsl = slice(gcol, gcol + CHUNK_COLS[n])
                      gcol += CHUNK_COLS[n]
                      nc.gpsimd.tensor_tensor(y_sb[:, sl], y_sb[:, sl],
                                              bias_rep_g[:, gsl],
                                              op=mybir.AluOpType.add)
              nc.sync.dma_start(y_flat[t * 128:(t + 1) * 128, :], y_sb[:])

      if rep_cm is not None:
          rep_cm.__exit__(None, None, None)

    _split_multi_waits(nc)
    return nc


def _get_nc(repeat=1):
    if repeat not in _CACHE:
        _CACHE[repeat] = _build(repeat)
    return _CACHE[repeat]


def kernel(x: np.ndarray, weight: np.ndarray, bias: np.ndarray,
           _trace: bool = False, _repeat: int = 1):
    nc = _get_nc(_repeat)
    xb = np.ascontiguousarray(x, dtype=np.float32).astype(ml_dtypes.bfloat16)
    wb = np.ascontiguousarray(weight, dtype=np.float32).astype(ml_dtypes.bfloat16)
    bb = np.ascontiguousarray(
        bias, dtype=np.float32).reshape(1, CO).astype(ml_dtypes.bfloat16)
    in_maps = [
        {"x": xb[i * BS:(i + 1) * BS], "weight": wb, "bias": bb}
        for i in range(N_CORES)
    ]
    res = run_bass_kernel_spmd(nc, in_maps, list(range(N_CORES)), trace=_trace)
    out = np.concatenate(
        [np.asarray(res.results[i]["y"]) for i in range(N_CORES)],
        axis=0).astype(np.float32)
    if _trace:
        return out, res
    return out
